# revision 47
# baseline (speedup 1.0000x reference)
"""Distributed Trainium2 kernel for the gated-adapter attention module.

Head-parallel tensor parallelism over 8 NeuronCores (4 heads each).
Host-side prep (inside kernel()): inputs are pre-transposed and
pre-cast to bf16 so the device never transposes weights or x —
xT [D, TOK], wqT/wkT/wvT [D, CH], and the core's wo column-slice
woT [CH, D] arrive matmul-ready.  The adapter K/V projections
(10x4096 @ 4096x512) and tanh(gate) are precomputed on host.

Device pipeline per core:
  A) QKV: x-stationary matmuls emit q/k/v natural [tok, ch]; RoPE on
     DVE straight out of PSUM; q/k PE-transposed to [ch, tok] and
     spilled to DRAM (contiguous loads later), v spilled natural.
  B) Attention per (batch, head) in S^T orientation: scores [k, q],
     uniform exp on ACT, multiplicative 0/1 diagonal masks on DVE,
     4:1 DVE pre-reduction then softmax sums via ones-matmul, PV
     accumulation, gated adapter branch; o^T accumulates in SBUF.
  C) RowParallel wo in y^T orientation: yT[dout, tok] = woT.T @ o^T
     with woT stationary (8 PSUM banks per dout-tile), partials
     ReduceScattered over dout in 4 chunks (pipelined against the
     matmuls); final f32 via SWDGE cast-DMA; host re-transposes.

Engine map: sync = x prefetch only; scalar = weights + compute-paced
copies/spills/exp; gpsimd = phase-B loads + RS triggers + output
(fire on data semaphores, never queue behind compute).
"""

import sys

sys.path.insert(0, "/opt/trn_rl_repo")

import numpy as np
import ml_dtypes

import concourse.bass as bass
import concourse.mybir as mybir
import concourse.tile as tile
from concourse import bacc, bass_utils
from concourse.bass import ds, ts
from concourse.masks import make_identity

N_CORES = 8
B, S, D = 2, 2048, 4096
H = 32
HD = 128                      # head dim
H_LOC = H // N_CORES          # 4 heads per core
CH = H_LOC * HD               # 512 local channels
TOK = B * S                   # 4096 tokens
NK = D // 128                 # 32 contraction tiles
AL = 10                       # adapter length
NQC = S // 512                # 4 query chunks per sequence
NDQ = 4                       # dout chunks for the wo/RS pipeline
DQW = D // NDQ                # 1024 dout rows per chunk
SCALE = 1.0 / float(np.sqrt(HD))
BF = mybir.dt.bfloat16
F32 = mybir.dt.float32
EXP = mybir.ActivationFunctionType.Exp
COPY = mybir.ActivationFunctionType.Copy
MULT = mybir.AluOpType.mult
BF_NP = ml_dtypes.bfloat16


def build():
    nc = bacc.Bacc("TRN2", target_bir_lowering=False, debug=False,
                   num_devices=N_CORES)
    xT = nc.dram_tensor("xT", [D, TOK], BF, kind="ExternalInput")
    wqT = nc.dram_tensor("wqT", [D, CH], BF, kind="ExternalInput")
    wkT = nc.dram_tensor("wkT", [D, CH], BF, kind="ExternalInput")
    wvT = nc.dram_tensor("wvT", [D, CH], BF, kind="ExternalInput")
    woTF = nc.dram_tensor("woTF", [D, D], BF, kind="ExternalInput")
    fcos = nc.dram_tensor("fcos", [S, HD // 2], BF, kind="ExternalInput")
    fsin = nc.dram_tensor("fsin", [S, HD // 2], BF, kind="ExternalInput")
    akT = nc.dram_tensor("akT", [HD, H_LOC * AL], BF, kind="ExternalInput")
    av = nc.dram_tensor("av", [AL, CH], BF, kind="ExternalInput")
    tg = nc.dram_tensor("tg", [1, H_LOC], F32, kind="ExternalInput")
    m01 = nc.dram_tensor("m01", [128, 4 * 512], BF, kind="ExternalInput")
    out = nc.dram_tensor("out", [TOK // N_CORES, D], F32,
                         kind="ExternalOutput")

    with tile.TileContext(nc) as tc:
        with tc.tile_pool(name="dram", bufs=1, space="DRAM") as dram, \
             tc.tile_pool(name="persist", bufs=1) as persist:
            at_cm = tc.tile_pool(name="at", bufs=3)
            at = at_cm.__enter__()
            qT_d = dram.tile([CH, TOK], BF, tag="qT_d")
            kT_d = dram.tile([CH, TOK], BF, tag="kT_d")
            v_d = dram.tile([TOK, CH], BF, tag="v_d")
            a2a_ins = [dram.tile([N_CORES, CH, 256], BF, tag=f"a2ai{b}",
                                 name=f"a2ai{b}") for b in range(B)]
            a2a_outs = [dram.tile([N_CORES, CH, 256], BF, tag=f"a2ao{b}",
                                  name=f"a2ao{b}") for b in range(B)]

            oTf = persist.tile([128, NK, TOK // N_CORES], BF, tag="oTf")
            # Phase B/C loads on the otherwise-idle gpsimd queue: they are
            # issued up front in its FIFO and fire as soon as the producing
            # spills' semaphores allow, prefetching across phase boundaries.
            def _bh_loads(b_i, h):
                qTb = at.tile([128, S], BF, tag="qTb", name="qTb")
                nc.gpsimd.dma_start(
                    qTb[:], qT_d[ds(h * HD, HD), ds(b_i * S, S)])
                kTb = at.tile([128, S], BF, tag="kTb", name="kTb")
                nc.gpsimd.dma_start(
                    kTb[:], kT_d[ds(h * HD, HD), ds(b_i * S, S)])
                vb2 = at.tile([128, S // 128, HD], BF, tag="vb2", name="vb2")
                nc.gpsimd.dma_start(
                    vb2[:],
                    v_d[ds(b_i * S, S), ts(h, HD)].rearrange(
                        "(kt p) d -> p kt d", p=128))
                return qTb, kTb, vb2

            # ================= phase A: QKV =================
            NPA = 16                  # 256-token panels
            with tc.tile_pool(name="wres", bufs=1) as wres, \
                 tc.tile_pool(name="xa", bufs=5) as xa, \
                 tc.tile_pool(name="ar", bufs=3) as ar, \
                 tc.tile_pool(name="aspill", bufs=2) as aspill, \
                 tc.tile_pool(name="ps_a", bufs=2, space="PSUM") as ps_a, \
                 tc.tile_pool(name="ps_t", bufs=2, space="PSUM") as ps_t:
                ident = persist.tile([128, 128], BF, tag="ident")
                make_identity(nc, ident[:])
                ones = persist.tile([128, 128], BF, tag="ones")
                nc.vector.memset(ones[:], 1.0)
                tg_sb = persist.tile([128, H_LOC], F32, tag="tg_sb")
                nc.scalar.dma_start(tg_sb[:], tg.ap().partition_broadcast(128))
                akT_sb = persist.tile([128, H_LOC, AL], BF, tag="akT_sb")
                nc.scalar.dma_start(
                    akT_sb[:], akT.ap().rearrange("p (h a) -> p h a", h=H_LOC))
                av_sb = persist.tile([AL, CH], BF, tag="av_sb")
                nc.scalar.dma_start(av_sb[:], av.ap())
                m01_sb = persist.tile([128, 4, 512], BF, tag="m01_sb")
                nc.scalar.dma_start(
                    m01_sb[:], m01.ap().rearrange("p (j q) -> p j q", j=4))
                # RoPE tables in the baseline layout: [128 part, S//128, 64]
                cs_all = persist.tile([128, S // 128, HD // 2], BF, tag="cs_all")
                nc.scalar.dma_start(
                    cs_all[:], fcos.ap().rearrange("(pb p) f -> p pb f", p=128))
                sn_all = persist.tile([128, S // 128, HD // 2], BF, tag="sn_all")
                nc.scalar.dma_start(
                    sn_all[:], fsin.ap().rearrange("(pb p) f -> p pb f", p=128))
                wTs = []
                for nm, wt in (("q", wqT), ("k", wkT), ("v", wvT)):
                    wT = wres.tile([128, NK, CH], BF, tag=f"wT{nm}",
                                   name=f"wT{nm}")
                    wTs.append(wT)
                # chunked weight loads: the first QKV chains depend only on
                # the leading contraction tiles, so the PE starts ~25us
                # earlier than with monolithic 4MB loads
                for cq in range(4):
                    for wT, wt in zip(wTs, (wqT, wkT, wvT)):
                        nc.scalar.dma_start(
                            wT[:, ds(cq * (NK // 4), NK // 4), :],
                            wt.ap()[ds(cq * (D // 4), D // 4), :].rearrange(
                                "(nk p) c -> p nk c", p=128))

                for pan in range(NPA):
                    # xT quarter-tiles for this panel: [128, 8, 256] each
                    xh = []
                    for qd in range(4):
                        xt = xa.tile([128, NK // 4, 256], BF, tag="xt")
                        nc.sync.dma_start(
                            xt[:],
                            xT.ap()[ds(qd * (D // 4), D // 4),
                                    ds(pan * 256, 256)].rearrange(
                                "(k p) t -> p k t", p=128))
                        xh.append(xt)
                    for ck in range(2):          # 128-token chunks in panel
                        tglob = pan * 2 + ck     # global token tile
                        srow = (tglob % (S // 128)) * 128  # within batch
                        csb = cs_all[:, srow // 128, :]
                        ssb = sn_all[:, srow // 128, :]
                        for p_i in range(3):     # chain-major: one PSUM bank
                            pp = ps_a.tile([128, CH], F32, tag=f"pp{p_i}",
                                           name=f"pp{p_i}")
                            for dt in range(NK):
                                lx = xh[dt // (NK // 4)][:, dt % (NK // 4),
                                                         ts(ck, 128)]
                                nc.tensor.matmul(
                                    pp[:], lhsT=lx, rhs=wTs[p_i][:, dt, :],
                                    start=(dt == 0), stop=(dt == NK - 1))
                            if p_i == 2:
                                # v: cast + spill natural
                                vb = ar.tile([128, CH], BF, tag="vb")
                                nc.scalar.activation(vb[:], pp[:], COPY)
                                nc.scalar.dma_start(
                                    v_d[ds(tglob * 128, 128), :], vb[:])
                                continue
                            # q, k: RoPE from PSUM -> natural bf16
                            dst = qT_d if p_i == 0 else kT_d
                            rp = ar.tile([128, CH], BF, tag=f"rp{p_i}",
                                         name=f"rp{p_i}")
                            for h in range(H_LOC):
                                pv2 = pp[:, ts(h, HD)].rearrange(
                                    "p (i two) -> p two i", two=2)
                                rv = rp[:, ts(h, HD)].rearrange(
                                    "p (i two) -> p two i", two=2)
                                a0, b0 = pv2[:, 0, :], pv2[:, 1, :]
                                t1 = ar.tile([128, HD // 2], F32, tag="t1")
                                t2 = ar.tile([128, HD // 2], F32, tag="t2")
                                nc.vector.tensor_mul(t1[:], a0, csb)
                                nc.vector.tensor_mul(t2[:], b0, ssb)
                                nc.vector.tensor_sub(rv[:, 0, :], t1[:], t2[:])
                                nc.vector.tensor_mul(t1[:], a0, ssb)
                                nc.vector.tensor_mul(t2[:], b0, csb)
                                nc.vector.tensor_add(rv[:, 1, :], t1[:], t2[:])
                            # transpose each 128-ch block to [ch, tok]
                            for ct in range(4):
                                tp = ps_t.tile([128, 128], BF, tag="tp")
                                nc.tensor.transpose(tp[:], rp[:, ts(ct, 128)],
                                                    ident[:])
                                st = aspill.tile([128, 128], BF, tag="st",
                                                 bufs=3)
                                nc.scalar.activation(st[:], tp[:], COPY)
                                nc.scalar.dma_start(
                                    dst[ds(ct * 128, 128),
                                        ds(tglob * 128, 128)], st[:])

            # ================= phase B: attention =================
            with tc.tile_pool(name="att", bufs=3) as att, \
                 tc.tile_pool(name="ps_st", bufs=4, space="PSUM") as ps_st, \
                 tc.tile_pool(name="ps_ac", bufs=1, space="PSUM") as ps_ac:
                all_loads = [_bh_loads(*divmod(bh, H_LOC))
                             for bh in range(B * H_LOC)]

                def emit_zip(fronts, backs):
                    for i in range(max(len(fronts), len(backs))):
                        if i < len(backs):
                            backs[i]()
                        if i < len(fronts):
                            fronts[i]()

                pend = []
                for bh in range(B * H_LOC):
                    b_i, h = divmod(bh, H_LOC)
                    qTb, kTb, vb2 = all_loads[bh]
                    for qc in range(NQC):
                        nkt = (qc + 1) * 4
                        stb = att.tile([128, S // 128, 512], BF, tag="stb",
                                       bufs=3)
                        sadd = att.tile([128, 2 * NQC, 512], BF, tag="sadd",
                                        bufs=3)

                        def _score(kt, stb=stb, qTb=qTb, kTb=kTb, qc=qc):
                            sps = ps_st.tile([128, 512], F32, tag="sps")
                            nc.tensor.matmul(sps[:],
                                             lhsT=kTb[:, ts(kt, 128)],
                                             rhs=qTb[:, ts(qc, 512)],
                                             start=True, stop=True)
                            nc.scalar.activation(stb[:, kt, :], sps[:],
                                                 EXP, scale=SCALE)
                            if kt // 4 == qc:
                                nc.vector.tensor_mul(
                                    stb[:, kt, :], stb[:, kt, :],
                                    m01_sb[:, kt % 4, :])

                        fronts = [lambda kt=kt, f=_score: f(kt)
                                  for kt in range(nkt)]
                        emit_zip(fronts, pend)

                        s2 = ps_ac.tile([33, 512], F32, tag="s2")
                        o_ps = ps_ac.tile([128, 512], F32, tag="o_ps", bufs=2)

                        def _pv(kt, stb=stb, sadd=sadd, vb2=vb2, o_ps=o_ps,
                                s2=s2, nkt=nkt):
                            nc.tensor.matmul(o_ps[:], lhsT=vb2[:, kt, :],
                                             rhs=stb[:, kt, :],
                                             start=(kt == 0),
                                             stop=(kt == nkt - 1))
                            if kt % 4 == 3:
                                g = kt // 4
                                nc.vector.tensor_add(
                                    sadd[:, 2 * g:2 * g + 2, :].rearrange(
                                        "p b a -> p (b a)"),
                                    stb[:, 4 * g:4 * g + 2, :].rearrange(
                                        "p b a -> p (b a)"),
                                    stb[:, 4 * g + 2:4 * g + 4, :].rearrange(
                                        "p b a -> p (b a)"))
                                nc.tensor.matmul(s2[0:1, :],
                                                 lhsT=ones[:, 0:1],
                                                 rhs=sadd[:, 2 * g, :],
                                                 start=(g == 0), stop=False)
                                nc.tensor.matmul(s2[0:1, :],
                                                 lhsT=ones[:, 0:1],
                                                 rhs=sadd[:, 2 * g + 1, :],
                                                 start=False,
                                                 stop=(g == nkt // 4 - 1))

                        def _adp1(qTb=qTb, qc=qc, h=h, s2=s2):
                            spa = ps_st.tile([128, 512], F32, tag="sps")
                            nc.tensor.matmul(spa[:AL, :],
                                             lhsT=akT_sb[:, h, :],
                                             rhs=qTb[:, ts(qc, 512)],
                                             start=True, stop=True)
                            pab = att.tile([AL, 512], BF, tag="pab")
                            nc.scalar.activation(pab[:], spa[:AL, :], EXP,
                                                 scale=SCALE)
                            _adp1.pab = pab

                        def _adp2(h=h, s2=s2, adp1_ref=_adp1):
                            pab = adp1_ref.pab
                            nc.tensor.matmul(s2[32:33, :],
                                             lhsT=ones[:AL, 0:1],
                                             rhs=pab[:], start=True,
                                             stop=True)
                            oa_ps = ps_ac.tile([128, 512], F32, tag="oa_ps")
                            nc.tensor.matmul(oa_ps[:],
                                             lhsT=av_sb[:, ts(h, HD)],
                                             rhs=pab[:], start=True,
                                             stop=True)
                            _adp2.oa_ps = oa_ps

                        def _cmb1(s2=s2):
                            rs2f = att.tile([33, 512], F32, tag="rs2f")
                            nc.vector.reciprocal_approx_fast(rs2f[:], s2[:])
                            rs2 = att.tile([33, 512], BF, tag="rs2")
                            nc.vector.tensor_copy(rs2[:], rs2f[:])
                            _cmb1.rs2 = rs2

                        def _cmb2(cmb1_ref=_cmb1):
                            rs2 = cmb1_ref.rs2
                            bc_ps = ps_st.tile([128, 512], F32, tag="sps")
                            bca_ps = ps_st.tile([128, 512], F32, tag="sps")
                            nc.tensor.matmul(bc_ps, lhsT=ones[0:1, :],
                                             rhs=rs2[0:1, :], start=True,
                                             stop=True)
                            nc.tensor.matmul(bca_ps, lhsT=ones[32:33, :],
                                             rhs=rs2[32:33, :], start=True,
                                             stop=True)
                            bcs = att.tile([128, 512], F32, tag="bcs")
                            nc.scalar.activation(bcs[:], bc_ps, COPY)
                            bcas = att.tile([128, 512], F32, tag="bcas")
                            nc.vector.tensor_copy(bcas[:], bca_ps)
                            _cmb2.bcs, _cmb2.bcas = bcs, bcas

                        def _fin(b_i=b_i, qc=qc, h=h, o_ps=o_ps,
                                 adp2_ref=_adp2, cmb2_ref=_cmb2):
                            bcs, bcas = cmb2_ref.bcs, cmb2_ref.bcas
                            oa_ps = adp2_ref.oa_ps
                            t3 = att.tile([128, 512], F32, tag="t3")
                            nc.vector.tensor_mul(t3[:], o_ps[:], bcs[:])
                            t4 = att.tile([128, 512], F32, tag="t4")
                            nc.vector.scalar_tensor_tensor(
                                t4[:], bcas[:], tg_sb[:, ds(h, 1)], oa_ps[:],
                                op0=MULT, op1=MULT)
                            ob = att.tile([128, 512], BF, tag="ob")
                            nc.vector.tensor_add(ob[:], t3[:], t4[:])
                            nc.scalar.dma_start(
                                a2a_ins[b_i][2 * qc][ds(h * HD, HD), :],
                                ob[:, 0:256])
                            nc.scalar.dma_start(
                                a2a_ins[b_i][2 * qc + 1][ds(h * HD, HD), :],
                                ob[:, 256:512])

                        pend = ([lambda kt=kt, f=_pv: f(kt)
                                 for kt in range(nkt)]
                                + [_adp1, _adp2, _cmb1, _cmb2, _fin])
                    if h == H_LOC - 1:
                        # drain the pipeline, then exchange this batch
                        emit_zip([], pend)
                        pend = []
                        nc.gpsimd.collective_compute(
                            "AllToAll", mybir.AluOpType.bypass,
                            replica_groups=[list(range(N_CORES))],
                            ins=[a2a_ins[b_i].opt()],
                            outs=[a2a_outs[b_i].opt()])
                        for sc in range(N_CORES):
                            nc.gpsimd.dma_start(
                                oTf[:, ds(sc * H_LOC, H_LOC),
                                    ds(b_i * 256, 256)],
                                a2a_outs[b_i][sc].rearrange(
                                    "(c p) t -> p c t", p=128))

            at_cm.__exit__(None, None, None)
            # ========== phase C: full-wo quarters ==========
            with tc.tile_pool(name="wof", bufs=2) as wof, \
                 tc.tile_pool(name="wy", bufs=4) as wy, \
                 tc.tile_pool(name="ps_y", bufs=2, space="PSUM") as ps_y:
                wqts = {}

                def _load_wqt(dq):
                    wqt = wof.tile([128, NK, DQW], BF, tag="wqt")
                    nch = 16 if dq == 0 else 4
                    for cq in range(nch):
                        nc.sync.dma_start(
                            wqt[:, ds(cq * (NK // nch), NK // nch), :],
                            woTF.ap()[ds(cq * (D // nch), D // nch),
                                      ds(dq * DQW, DQW)].rearrange(
                                "(ct p) d -> p ct d", p=128))
                    wqts[dq] = wqt

                def _chain(dq, tt):
                    wqt = wqts[dq]
                    yt = ps_y.tile([128, DQW], F32, tag="yt")
                    for ct in range(NK):
                        for dc in range(DQW // 512):
                            nc.tensor.matmul(
                                yt[:, ts(dc, 512)],
                                lhsT=oTf[:, ct, ts(tt, 128)],
                                rhs=wqt[:, ct, ts(dc, 512)],
                                start=(ct == 0), stop=(ct == NK - 1))
                    yf = wy.tile([128, DQW], F32, tag="yf")
                    nc.scalar.activation(yf[:], yt[:], COPY)
                    nc.scalar.dma_start(
                        out.ap()[ds(tt * 128, 128), ds(dq * DQW, DQW)],
                        yf[:])

                # b0-token chains of dq0/dq1 first so the b1 AllToAll
                # (skew + exchange) hides under them
                _load_wqt(0)
                _load_wqt(1)
                for dq, tt in ((0, 0), (0, 1), (1, 0), (1, 1),
                               (0, 2), (0, 3), (1, 2), (1, 3)):
                    _chain(dq, tt)
                for dq in (2, 3):
                    _load_wqt(dq)
                    for tt in range(4):
                        _chain(dq, tt)
    nc.compile()
    return nc


_NC_CACHE = None


def _prep(x, wq, wk, wv, wo, gate, adapter, freqs_cos, freqs_sin, mask):
    """Host-side layout prep. Returns per-core input maps."""
    xf = np.asarray(x, np.float32).reshape(TOK, D)
    xT = np.ascontiguousarray(xf.T).astype(BF_NP)
    wq = np.asarray(wq, np.float32)
    wk = np.asarray(wk, np.float32)
    wv = np.asarray(wv, np.float32)
    wo = np.asarray(wo, np.float32)
    g = np.tanh(np.asarray(gate, np.float32).reshape(H))
    ad = np.asarray(adapter, np.float32).reshape(AL, D)
    a_k = ad @ wk.T          # [AL, H*HD]
    a_v = ad @ wv.T
    fc = np.ascontiguousarray(np.asarray(freqs_cos, np.float32)).astype(BF_NP)
    fs = np.ascontiguousarray(np.asarray(freqs_sin, np.float32)).astype(BF_NP)
    woTF = np.ascontiguousarray(wo.T).astype(BF_NP)
    mk = np.asarray(mask, np.float32).reshape(S, S)
    # multiplicative 0/1 diagonal masks, S^T orientation: m01[j][k, q]
    m01 = np.empty((128, 4, 512), np.float32)
    for j in range(4):
        blk = mk[0:512, j * 128:(j + 1) * 128]    # [q, k] additive
        m01[:, j, :] = (blk == 0.0).T.astype(np.float32)
    m01 = np.ascontiguousarray(m01.reshape(128, 4 * 512)).astype(BF_NP)

    in_maps = []
    for r in range(N_CORES):
        sl = slice(r * CH, (r + 1) * CH)
        akr = a_k[:, sl]     # [AL, CH]
        akT = np.zeros((HD, H_LOC, AL), np.float32)
        for h in range(H_LOC):
            akT[:, h, :] = akr[:, h * HD:(h + 1) * HD].T
        in_maps.append({
            "xT": xT,
            "wqT": np.ascontiguousarray(wq[sl].T).astype(BF_NP),
            "wkT": np.ascontiguousarray(wk[sl].T).astype(BF_NP),
            "wvT": np.ascontiguousarray(wv[sl].T).astype(BF_NP),
            "woTF": woTF,
            "fcos": fc,
            "fsin": fs,
            "akT": np.ascontiguousarray(
                akT.reshape(HD, H_LOC * AL)).astype(BF_NP),
            "av": np.ascontiguousarray(a_v[:, sl]).astype(BF_NP),
            "tg": np.ascontiguousarray(
                g[r * H_LOC:(r + 1) * H_LOC].reshape(1, H_LOC)),
            "m01": m01,
        })
    return in_maps


def kernel(x, wq, wk, wv, wo, gate, adapter, freqs_cos, freqs_sin, mask,
           start_pos=0, **_unused):
    global _NC_CACHE
    if _NC_CACHE is None:
        _NC_CACHE = build()
    nc = _NC_CACHE
    in_maps = _prep(x, wq, wk, wv, wo, gate, adapter,
                    freqs_cos, freqs_sin, mask)
    res = bass_utils.run_bass_kernel_spmd(nc, in_maps,
                                          core_ids=list(range(N_CORES)))
    # core r holds b0 tokens [r*256, r*256+256) then b1 tokens likewise
    y = np.empty((TOK, D), np.float32)
    for r in range(N_CORES):
        arr = np.asarray(res.results[r]["out"])
        y[r * 256:(r + 1) * 256] = arr[0:256]
        y[S + r * 256:S + (r + 1) * 256] = arr[256:512]
    return y.reshape(B, S, D)


if __name__ == "__main__":
    nc = build()
    print("compiled ok, instrs:",
          sum(len(bb.instructions) for f in nc.m.functions for bb in f.blocks))
